# revision 1
# baseline (speedup 1.0000x reference)
"""Trainium2 Bass kernel for the DTGL GCN+windowed-LSTM module.

Computation (see reference):
  h = relu(adj @ (x @ Wg0 + bg0));  h = relu(adj @ (h @ Wg1 + bg1))
  for p in 1..4: run LSTM_p over disjoint length-p windows of h (zero init
  state), writing the last hidden state back at each window end (in place).

Sharding: pure data-parallel over batch B=64 across 8 cores (8 batches per
core); adj and all weights replicated. No collectives.

Device algorithm per core (all matmuls fp32r = full PE rate):
  GCN bias reassociated: adj @ (x@W + 1 b^T) = (adj@x)@W + rowsum(adj) (x) b,
  so every matmul keeps its contraction dim on partitions with no activation
  transposes:
    1A: z1T[d,u] = sum_t x[t,d]*adjT[t,u]        (lhsT=x tile, rhs=adjT)
    1B: h1[u,h]  = relu(sum_d z1T[d,u]*Wg0[d,h] + rs[u]*bg0[h])
    2A: z2T[h,u] = sum_t h1[t,h]*adjT[t,u]
    2B: h2T[h,u] = relu(sum_k Wg1[k,h]*z2T[k,u] + bg1[h]*rs[u])
  h2T stays feature-major in SBUF; the 4 LSTM passes update it in place
  (gates via PSUM-accumulated matmuls, sigmoid/tanh on ACT with fused
  per-partition bias, cell math on DVE). Finally h2T is PE-transposed back
  to row-major and DMA'd out.

Batches run in 2 groups of 4 so the h1 + h2T slabs fit in SBUF.
"""

import numpy as np

B, T, D, H = 64, 2048, 256, 256
MAX_SKIP = 4
NCORES = 8
BL = B // NCORES          # batches per core
G = 4                     # batches per group
TK = T // 128             # 16 t-chunks
UC = T // 512             # 4 u-chunks of 512
HK = H // 128             # 2 feature blocks
JB0 = {"i": 0, "f": 2, "g": 4, "o": 6}

_COMPILED = None


def _build_program():
    import concourse.mybir as mybir
    import concourse.tile as tile
    from concourse import bacc

    f32 = mybir.dt.float32
    f32r = mybir.dt.float32r

    nc = bacc.Bacc("TRN2", target_bir_lowering=False, debug=False)

    io = dict(
        x=nc.dram_tensor("x", [BL, T, D], f32r, kind="ExternalInput").ap(),
        adjT=nc.dram_tensor("adjT", [T, T], f32r, kind="ExternalInput").ap(),
        rs=nc.dram_tensor("rs", [1, T], f32r, kind="ExternalInput").ap(),
        wg0=nc.dram_tensor("wg0", [D, H], f32r, kind="ExternalInput").ap(),
        wg1=nc.dram_tensor("wg1", [D, H], f32r, kind="ExternalInput").ap(),
        bg0=nc.dram_tensor("bg0", [1, H], f32r, kind="ExternalInput").ap(),
        bg1=nc.dram_tensor("bg1", [1, H], f32r, kind="ExternalInput").ap(),
        wihT=nc.dram_tensor("wihT", [MAX_SKIP, H, 4 * H], f32r, kind="ExternalInput").ap(),
        whhT=nc.dram_tensor("whhT", [MAX_SKIP, H, 4 * H], f32r, kind="ExternalInput").ap(),
        biasT=nc.dram_tensor("biasT", [128, MAX_SKIP * 8], f32, kind="ExternalInput").ap(),
        out=nc.dram_tensor("out", [BL, T, D], f32, kind="ExternalOutput").ap(),
    )

    with tile.TileContext(nc) as tc:
        _emit(nc, tc, mybir, io)

    nc.compile()
    return nc


def _emit(nc, tc, mybir, io):
    from contextlib import ExitStack
    from concourse.masks import make_identity

    f32 = mybir.dt.float32
    f32r = mybir.dt.float32r
    AF = mybir.ActivationFunctionType

    with ExitStack() as root:
        cp = root.enter_context(tc.tile_pool(name="const", bufs=1))
        wg0_sb = cp.tile([128, HK * H], f32r, name="wg0_sb")
        wg1_sb = cp.tile([128, HK * H], f32r, name="wg1_sb")
        for hk in range(HK):
            nc.sync.dma_start(out=wg0_sb[:, hk * H:(hk + 1) * H],
                              in_=io["wg0"][hk * 128:(hk + 1) * 128, :])
            nc.sync.dma_start(out=wg1_sb[:, hk * H:(hk + 1) * H],
                              in_=io["wg1"][hk * 128:(hk + 1) * 128, :])
        bg0_sb = cp.tile([1, H], f32r, name="bg0_sb")
        bg1_sb = cp.tile([1, H], f32r, name="bg1_sb")
        rs_sb = cp.tile([1, T], f32r, name="rs_sb")
        biasT_sb = cp.tile([128, MAX_SKIP * 8], f32, name="biasT_sb")
        nc.sync.dma_start(out=bg0_sb[:], in_=io["bg0"][:])
        nc.sync.dma_start(out=bg1_sb[:], in_=io["bg1"][:])
        nc.sync.dma_start(out=rs_sb[:], in_=io["rs"][:])
        nc.sync.dma_start(out=biasT_sb[:], in_=io["biasT"][:])
        id32 = cp.tile([128, 128], f32, name="id32")
        ident = cp.tile([128, 128], f32r, name="ident")
        make_identity(nc, id32[:])
        nc.vector.tensor_copy(ident[:], id32[:])

        h2t_pool = root.enter_context(tc.tile_pool(name="h2tp", bufs=1))
        lw_pool = root.enter_context(tc.tile_pool(name="lw", bufs=1))

        for grp in range(BL // G):
            _group(nc, tc, io, f32, f32r, AF, grp, wg0_sb, wg1_sb, bg0_sb,
                   bg1_sb, rs_sb, biasT_sb, ident, h2t_pool, lw_pool)


def _group(nc, tc, io, f32, f32r, AF, grp, wg0_sb, wg1_sb, bg0_sb, bg1_sb,
           rs_sb, biasT_sb, ident, h2t_pool, lw_pool):
    from contextlib import ExitStack

    bs = grp * G
    # h2T slabs: feature-major [j(part), hk*T + t], f32r, one per batch.
    h2t = [h2t_pool.tile([128, HK * T], f32r, name=f"h2t_{j}", tag=f"h2t_{j}")
           for j in range(G)]

    with ExitStack() as gcn:
        h1_pool = gcn.enter_context(tc.tile_pool(name="h1p", bufs=1))
        # h1 slabs: row-major [u(part), ublk*H + h], f32r.
        h1 = [h1_pool.tile([128, TK * H], f32r, name=f"h1_{j}", tag=f"h1_{j}")
              for j in range(G)]

        # ---------------- Phase 1: layer 1 (1A + 1B) ----------------
        with ExitStack() as ph:
            adjt_pool = ph.enter_context(tc.tile_pool(name="adjt", bufs=1))
            x_pool = ph.enter_context(tc.tile_pool(name="xp", bufs=6))
            z_pool = ph.enter_context(tc.tile_pool(name="zp", bufs=2))
            zps = ph.enter_context(tc.tile_pool(name="zps", bufs=4, space="PSUM"))
            mps = ph.enter_context(tc.tile_pool(name="mps", bufs=4, space="PSUM"))

            for u4 in range(UC):
                us = u4 * 512
                adjt = []
                for k in range(TK):
                    a = adjt_pool.tile([128, 512], f32r, name=f"adjt_{k}", tag=f"a_{k}")
                    nc.scalar.dma_start(out=a[:], in_=io["adjT"][k * 128:(k + 1) * 128, us:us + 512])
                    adjt.append(a)
                for j in range(G):
                    b = bs + j
                    z1t = z_pool.tile([128, HK * 512], f32r, name="z1t", tag="z1t")
                    zp = [zps.tile([128, 512], f32, name=f"zps{dk}", tag="zps_t")
                          for dk in range(HK)]
                    for k in range(TK):
                        xt = x_pool.tile([128, D], f32r, name="xt", tag="xt")
                        nc.sync.dma_start(out=xt[:], in_=io["x"][b, k * 128:(k + 1) * 128, :])
                        for dk in range(HK):
                            nc.tensor.matmul(zp[dk][:], xt[:, dk * 128:(dk + 1) * 128],
                                             adjt[k][:],
                                             start=(k == 0), stop=(k == TK - 1))
                    for dk in range(HK):
                        nc.vector.tensor_copy(z1t[:, dk * 512:(dk + 1) * 512], zp[dk][:])
                    for ub in range(4):
                        ua = us + ub * 128
                        hp = mps.tile([128, H], f32, name="hp", tag="mps_t")
                        for dk in range(HK):
                            nc.tensor.matmul(hp[:],
                                             z1t[:, dk * 512 + ub * 128: dk * 512 + (ub + 1) * 128],
                                             wg0_sb[:, dk * H:(dk + 1) * H],
                                             start=(dk == 0), stop=False)
                        nc.tensor.matmul(hp[:], rs_sb[0:1, ua:ua + 128], bg0_sb[0:1, :],
                                         start=False, stop=True)
                        nc.scalar.activation(h1[j][:, (u4 * 4 + ub) * H:(u4 * 4 + ub + 1) * H],
                                             hp[:], AF.Relu)

        # ---------------- Phase 2: layer 2 (2A + 2B) ----------------
        with ExitStack() as ph:
            adjt_pool = ph.enter_context(tc.tile_pool(name="adjt2", bufs=1))
            z_pool = ph.enter_context(tc.tile_pool(name="zp2", bufs=2))
            zps = ph.enter_context(tc.tile_pool(name="zps2", bufs=4, space="PSUM"))
            mps = ph.enter_context(tc.tile_pool(name="mps2", bufs=4, space="PSUM"))

            for u4 in range(UC):
                us = u4 * 512
                adjt = []
                for k in range(TK):
                    a = adjt_pool.tile([128, 512], f32r, name=f"adjt2_{k}", tag=f"a2_{k}")
                    nc.scalar.dma_start(out=a[:], in_=io["adjT"][k * 128:(k + 1) * 128, us:us + 512])
                    adjt.append(a)
                for j in range(G):
                    z2t = z_pool.tile([128, HK * 512], f32r, name="z2t", tag="z2t")
                    for hk in range(HK):
                        zp = zps.tile([128, 512], f32, name="zps2", tag="zps2_t")
                        for k in range(TK):
                            nc.tensor.matmul(zp[:],
                                             h1[j][:, k * H + hk * 128: k * H + (hk + 1) * 128],
                                             adjt[k][:],
                                             start=(k == 0), stop=(k == TK - 1))
                        nc.vector.tensor_copy(z2t[:, hk * 512:(hk + 1) * 512], zp[:])
                    for ho in range(HK):
                        hp = mps.tile([128, 512], f32, name="hp2", tag="mps2_t")
                        for hk in range(HK):
                            nc.tensor.matmul(hp[:],
                                             wg1_sb[:, hk * H + ho * 128: hk * H + (ho + 1) * 128],
                                             z2t[:, hk * 512:(hk + 1) * 512],
                                             start=(hk == 0), stop=False)
                        nc.tensor.matmul(hp[:], bg1_sb[0:1, ho * 128:(ho + 1) * 128],
                                         rs_sb[0:1, us:us + 512], start=False, stop=True)
                        nc.scalar.activation(h2t[j][:, ho * T + us: ho * T + us + 512],
                                             hp[:], AF.Relu)

    # ---------------- Phases 3-4: the four LSTM passes ----------------
    with ExitStack() as ph:
        gps = ph.enter_context(tc.tile_pool(name="gps", bufs=1, space="PSUM"))
        gsb = ph.enter_context(tc.tile_pool(name="gsb", bufs=2))
        st_pool = ph.enter_context(tc.tile_pool(name="st", bufs=4))
        gx_pool = ph.enter_context(tc.tile_pool(name="gx", bufs=3))

        for p in range(1, MAX_SKIP + 1):
            nw = T // p
            wih = lw_pool.tile([128, HK * 4 * H], f32r, name=f"wih{grp}{p}", tag="wih")
            for hk in range(HK):
                nc.sync.dma_start(out=wih[:, hk * 4 * H:(hk + 1) * 4 * H],
                                  in_=io["wihT"][p - 1, hk * 128:(hk + 1) * 128, :])
            whh = None
            if p > 1:
                whh = lw_pool.tile([128, HK * 4 * H], f32r, name=f"whh{grp}{p}", tag="whh")
                for hk in range(HK):
                    nc.sync.dma_start(out=whh[:, hk * 4 * H:(hk + 1) * 4 * H],
                                      in_=io["whhT"][p - 1, hk * 128:(hk + 1) * 128, :])

            for j in range(G):
                sl = h2t[j]
                view = [sl[:, hk * T: hk * T + nw * p].rearrange(
                    "a (w q) -> a w q", q=p) for hk in range(HK)]
                for ws in range(0, nw, 512):
                    ncw = min(512, nw - ws)
                    c_t = None
                    h_t = None
                    for t in range(p):
                        if p > 1:
                            xc = gx_pool.tile([128, 1024], f32r, name="xc", tag="xc")
                            for hk in range(HK):
                                nc.gpsimd.tensor_copy(
                                    xc[:, hk * 512: hk * 512 + ncw],
                                    view[hk][:, ws:ws + ncw, t:t + 1])
                        gates = "igo" if t == 0 else "ifgo"
                        gp = {}
                        for gn in gates:
                            psum = gps.tile([128, 1024], f32, name=f"ps_{gn}", tag=f"ps_{gn}")
                            gp[gn] = psum
                            for half in range(2):
                                jb = JB0[gn] + half
                                o = psum[:, half * 512: half * 512 + ncw]
                                for hk in range(HK):
                                    rhs = (view[hk][:, ws:ws + ncw, t:t + 1] if p == 1
                                           else xc[:, hk * 512: hk * 512 + ncw])
                                    nc.tensor.matmul(
                                        o,
                                        wih[:, hk * 4 * H + jb * 128: hk * 4 * H + (jb + 1) * 128],
                                        rhs,
                                        start=(hk == 0),
                                        stop=(t == 0 and hk == HK - 1))
                                if t > 0:
                                    for hk in range(HK):
                                        nc.tensor.matmul(
                                            o,
                                            whh[:, hk * 4 * H + jb * 128: hk * 4 * H + (jb + 1) * 128],
                                            h_t[:, hk * 512: hk * 512 + ncw],
                                            start=False, stop=(hk == HK - 1))
                        act = {}
                        for gn in gates:
                            fn = AF.Tanh if gn == "g" else AF.Sigmoid
                            a = gsb.tile([128, 1024], f32, name=f"a_{gn}", tag=f"a_{gn}")
                            act[gn] = a
                            for half in range(2):
                                col = (p - 1) * 8 + JB0[gn] + half
                                nc.scalar.activation(
                                    a[:, half * 512: half * 512 + ncw],
                                    gp[gn][:, half * 512: half * 512 + ncw],
                                    fn, bias=biasT_sb[:, col:col + 1])
                        n2 = [slice(0, ncw), slice(512, 512 + ncw)]
                        cn = st_pool.tile([128, 1024], f32, name="cn", tag="c")
                        if t == 0:
                            for s in n2:
                                nc.vector.tensor_mul(cn[:, s], act["i"][:, s], act["g"][:, s])
                        else:
                            # i*g computed in place into the g tile
                            for s in n2:
                                nc.vector.tensor_mul(act["g"][:, s], act["i"][:, s], act["g"][:, s])
                            for s in n2:
                                nc.vector.tensor_mul(cn[:, s], act["f"][:, s], c_t[:, s])
                            for s in n2:
                                nc.vector.tensor_add(cn[:, s], cn[:, s], act["g"][:, s])
                        c_t = cn
                        # tanh(c) overwrites the i tile (free after c update)
                        tc_t = act["i"]
                        for s in n2:
                            nc.scalar.activation(tc_t[:, s], c_t[:, s], AF.Tanh)
                        if t == p - 1:
                            for hk in range(HK):
                                nc.vector.tensor_mul(
                                    view[hk][:, ws:ws + ncw, p - 1:p],
                                    act["o"][:, hk * 512: hk * 512 + ncw],
                                    tc_t[:, hk * 512: hk * 512 + ncw])
                        else:
                            hn = st_pool.tile([128, 1024], f32r, name="hn", tag="h")
                            for s in n2:
                                nc.vector.tensor_mul(hn[:, s], act["o"][:, s], tc_t[:, s])
                            h_t = hn

    # ---------------- Phase 5: transpose h2T -> out ----------------
    with ExitStack() as ph:
        tps = ph.enter_context(tc.tile_pool(name="tps", bufs=4, space="PSUM"))
        osb = ph.enter_context(tc.tile_pool(name="osb", bufs=4))
        for j in range(G):
            b = bs + j
            for tk in range(TK):
                tp = tps.tile([128, D], f32r, name="tp", tag="tp")
                for hk in range(HK):
                    nc.tensor.transpose(tp[:, hk * 128:(hk + 1) * 128],
                                        h2t[j][:, hk * T + tk * 128: hk * T + (tk + 1) * 128],
                                        ident[:])
                ot = osb.tile([128, D], f32, name="ot", tag="ot")
                if tk % 2 == 0:
                    nc.scalar.activation(ot[:], tp[:], AF.Copy)
                else:
                    nc.vector.tensor_copy(ot[:], tp[:])
                nc.sync.dma_start(out=io["out"][b, tk * 128:(tk + 1) * 128, :], in_=ot[:])


def _prep_host(inputs):
    x = np.ascontiguousarray(inputs["x"], dtype=np.float32)
    adj = np.asarray(inputs["adj"], dtype=np.float32)
    adjT = np.ascontiguousarray(adj.T)
    rs = np.ascontiguousarray(adj.sum(axis=1, dtype=np.float32).reshape(1, T))
    wg0 = np.ascontiguousarray(inputs["Wg0"], dtype=np.float32)
    wg1 = np.ascontiguousarray(inputs["Wg1"], dtype=np.float32)
    bg0 = np.ascontiguousarray(inputs["bg0"], dtype=np.float32).reshape(1, H)
    bg1 = np.ascontiguousarray(inputs["bg1"], dtype=np.float32).reshape(1, H)
    wihT = np.ascontiguousarray(np.asarray(inputs["Wih"], dtype=np.float32).transpose(0, 2, 1))
    whhT = np.ascontiguousarray(np.asarray(inputs["Whh"], dtype=np.float32).transpose(0, 2, 1))
    bias = np.asarray(inputs["bih"], dtype=np.float32) + np.asarray(inputs["bhh"], dtype=np.float32)
    biasT = np.ascontiguousarray(
        bias.reshape(MAX_SKIP, 8, 128).transpose(2, 0, 1).reshape(128, MAX_SKIP * 8))
    shared = dict(adjT=adjT, rs=rs, wg0=wg0, wg1=wg1, bg0=bg0, bg1=bg1,
                  wihT=wihT, whhT=whhT, biasT=biasT)
    in_maps = []
    for c in range(NCORES):
        m = dict(shared)
        m["x"] = np.ascontiguousarray(x[c * BL:(c + 1) * BL])
        in_maps.append(m)
    return in_maps


def get_compiled():
    global _COMPILED
    if _COMPILED is None:
        _COMPILED = _build_program()
    return _COMPILED


def kernel(**inputs) -> np.ndarray:
    from concourse.bass_utils import run_bass_kernel_spmd

    nc = get_compiled()
    in_maps = _prep_host(inputs)
    res = run_bass_kernel_spmd(nc, in_maps, list(range(NCORES)))
    out = np.concatenate([res.results[c]["out"] for c in range(NCORES)], axis=0)
    return out.astype(np.float32)



# revision 11
# speedup vs baseline: 1.4116x; 1.4116x over previous
"""Trainium2 Bass kernel for the DTGL GCN+windowed-LSTM module (bf16 rewrite).

Computation (see reference):
  h = relu(adj @ (x @ Wg0 + bg0));  h = relu(adj @ (h @ Wg1 + bg1))
  for p in 1..4: run LSTM_p over disjoint length-p windows of h (zero init
  state), writing the last hidden state back at each window end (in place).

Sharding: pure data-parallel over batch B=64 across 8 cores (8 batches per
core); adj and all weights replicated. No collectives.

Perf design vs the fp32r baseline:
  - All matmul operands bf16 (PSUM accumulation stays f32): halves the
    moving-stream bytes, halves LDWEIGHTS time, halves SBUF/DMA footprint,
    and lowers PE power so the HAM clock-gate stays at full rate.
  - adjT is SBUF-resident (loaded once, bf16, 64KB/partition).
  - Software-pipelined phases: 1B(j-1) matmuls fill the PE pipe while
    1A(j)'s PSUM->SBUF copies drain (same for 2A/2B), so the PE never
    waits on a copy.
  - LSTM runs batches round-robin per timestep so PE matmuls of batch j+1
    overlap ACT/DVE/Pool cell math of batch j. Gate PSUM uses all 8 banks.
  - Output transpose via identity matmul (stationary = h2T block, moving =
    identity) producing f32 PSUM directly.
"""

import numpy as np

B, T, D, H = 64, 2048, 256, 256
MAX_SKIP = 4
NCORES = 8
BL = B // NCORES          # batches per core
G = 2                     # batches per group
NGRP = BL // G
TK = T // 128             # 16 t-chunks
JB0 = {"i": 0, "f": 2, "g": 4, "o": 6}

_COMPILED = None


def _build_program():
    import concourse.mybir as mybir
    import concourse.tile as tile
    from concourse import bacc

    f32 = mybir.dt.float32
    bf16 = mybir.dt.bfloat16

    nc = bacc.Bacc("TRN2", target_bir_lowering=False, debug=False)

    io = dict(
        x=nc.dram_tensor("x", [BL, T, D], bf16, kind="ExternalInput").ap(),
        adjT=nc.dram_tensor("adjT", [T, T], bf16, kind="ExternalInput").ap(),
        rs=nc.dram_tensor("rs", [1, T], bf16, kind="ExternalInput").ap(),
        wg0=nc.dram_tensor("wg0", [D, H], bf16, kind="ExternalInput").ap(),
        wg1=nc.dram_tensor("wg1", [D, H], bf16, kind="ExternalInput").ap(),
        bg0=nc.dram_tensor("bg0", [1, H], bf16, kind="ExternalInput").ap(),
        bg1=nc.dram_tensor("bg1", [1, H], bf16, kind="ExternalInput").ap(),
        wihT=nc.dram_tensor("wihT", [MAX_SKIP, H, 4 * H], bf16, kind="ExternalInput").ap(),
        whhT=nc.dram_tensor("whhT", [MAX_SKIP, H, 4 * H], bf16, kind="ExternalInput").ap(),
        biasT=nc.dram_tensor("biasT", [128, MAX_SKIP * 8], f32, kind="ExternalInput").ap(),
        out=nc.dram_tensor("out", [BL, T, D], f32, kind="ExternalOutput").ap(),
    )

    with tile.TileContext(nc) as tc:
        _emit(nc, tc, mybir, io)

    nc.compile()
    return nc


def _emit(nc, tc, mybir, io):
    from contextlib import ExitStack
    from concourse.masks import make_identity

    f32 = mybir.dt.float32
    bf16 = mybir.dt.bfloat16
    AF = mybir.ActivationFunctionType

    with ExitStack() as root:
        cp = root.enter_context(tc.tile_pool(name="const", bufs=1))
        # adjT resident: 16 tiles [128, 2048] bf16
        adjt = []
        for k in range(TK):
            a = cp.tile([128, T], bf16, name=f"adjt_{k}")
            nc.sync.dma_start(out=a[:], in_=io["adjT"][k * 128:(k + 1) * 128, :])
            adjt.append(a)
        wg0_sb = cp.tile([128, 2 * H], bf16, name="wg0_sb")
        wg1_sb = cp.tile([128, 2 * H], bf16, name="wg1_sb")
        for dk in range(2):
            nc.sync.dma_start(out=wg0_sb[:, dk * H:(dk + 1) * H],
                              in_=io["wg0"][dk * 128:(dk + 1) * 128, :])
            nc.sync.dma_start(out=wg1_sb[:, dk * H:(dk + 1) * H],
                              in_=io["wg1"][dk * 128:(dk + 1) * 128, :])
        bg0_sb = cp.tile([1, H], bf16, name="bg0_sb")
        bg1_sb = cp.tile([1, H], bf16, name="bg1_sb")
        rs_sb = cp.tile([1, T], bf16, name="rs_sb")
        biasT_sb = cp.tile([128, MAX_SKIP * 8], f32, name="biasT_sb")
        nc.sync.dma_start(out=bg0_sb[:], in_=io["bg0"][:])
        nc.sync.dma_start(out=bg1_sb[:], in_=io["bg1"][:])
        nc.sync.dma_start(out=rs_sb[:], in_=io["rs"][:])
        nc.sync.dma_start(out=biasT_sb[:], in_=io["biasT"][:])
        id32 = cp.tile([128, 128], f32, name="id32")
        ident = cp.tile([128, 128], bf16, name="ident")
        make_identity(nc, id32[:])
        nc.vector.tensor_copy(ident[:], id32[:])

        h2t_pool = root.enter_context(tc.tile_pool(name="h2tp", bufs=1))
        lw_pool = root.enter_context(tc.tile_pool(name="lw", bufs=2))

        for grp in range(NGRP):
            _group(nc, tc, io, f32, bf16, AF, grp, adjt, wg0_sb, wg1_sb,
                   bg0_sb, bg1_sb, rs_sb, biasT_sb, ident, h2t_pool, lw_pool)


def _group(nc, tc, io, f32, bf16, AF, grp, adjt, wg0_sb, wg1_sb, bg0_sb,
           bg1_sb, rs_sb, biasT_sb, ident, h2t_pool, lw_pool):
    from contextlib import ExitStack

    bs = grp * G
    # h2T slabs: feature-major [h(part within hk), hk*T + t], bf16.
    h2t = [h2t_pool.tile([128, 2 * T], bf16, name=f"h2t_{j}", tag=f"h2t_{j}")
           for j in range(G)]

    with ExitStack() as gcn:
        h1_pool = gcn.enter_context(tc.tile_pool(name="h1p", bufs=1))
        # h1 slabs: row-major [u(part within ub), ub*H + h], bf16.
        h1 = [h1_pool.tile([128, TK * H], bf16, name=f"h1_{j}", tag=f"h1_{j}")
              for j in range(G)]
        x_pool = gcn.enter_context(tc.tile_pool(name="xp", bufs=2))
        z1_pool = gcn.enter_context(tc.tile_pool(name="z1p", bufs=2))

        # ---------------- Phase 1: layer 1 (1A + 1B pipelined) ----------------
        with ExitStack() as ph:
            zps = ph.enter_context(tc.tile_pool(name="zps", bufs=1, space="PSUM"))
            hps = ph.enter_context(tc.tile_pool(name="hps", bufs=4, space="PSUM"))

            xs = []
            for j in range(G):
                xt = x_pool.tile([128, TK * D], bf16, name=f"x_{j}", tag="xs")
                nc.sync.dma_start(
                    out=xt[:].rearrange("p (k d) -> p k d", d=D),
                    in_=io["x"][bs + j].rearrange("(k p) d -> p k d", p=128))
                xs.append(xt)

            z1t = {}   # (j, uh, dk) -> sbuf tile [128, 1024] bf16
            pend = []  # queue of emitted-1A halves awaiting 1B: (j, uh)

            def emit_1a(j, uh):
                zp = {(dk, q): zps.tile([128, 512], f32, name="zp", tag=f"zp{dk}{q}")
                      for dk in range(2) for q in range(2)}
                for k in range(TK):
                    for dk in range(2):
                        lhs = xs[j][:, k * D + dk * 128: k * D + (dk + 1) * 128]
                        for q in range(2):
                            nc.tensor.matmul(
                                zp[(dk, q)][:], lhs,
                                adjt[k][:, uh * 1024 + q * 512: uh * 1024 + (q + 1) * 512],
                                start=(k == 0), stop=(k == TK - 1))
                for dk in range(2):
                    zt = z1_pool.tile([128, 1024], bf16, name="z1t", tag=f"z1t{dk}")
                    z1t[(j, uh, dk)] = zt
                    nc.vector.tensor_copy(zt[:, 0:512], zp[(dk, 0)][:])
                    nc.vector.tensor_copy(zt[:, 512:1024], zp[(dk, 1)][:])

            def emit_1b(j, uh):
                for ub_l in range(8):
                    ub = uh * 8 + ub_l
                    hp = hps.tile([128, H], f32, name="hp", tag="hp")
                    for dk in range(2):
                        nc.tensor.matmul(
                            hp[:], z1t[(j, uh, dk)][:, ub_l * 128:(ub_l + 1) * 128],
                            wg0_sb[:, dk * H:(dk + 1) * H],
                            start=(dk == 0), stop=False)
                    nc.tensor.matmul(hp[:], rs_sb[0:1, ub * 128:(ub + 1) * 128],
                                     bg0_sb[0:1, :], start=False, stop=True)
                    nc.vector.tensor_relu(h1[j][:, ub * H:(ub + 1) * H], hp[:])

            for j in range(G):
                for uh in range(2):
                    emit_1a(j, uh)
                    pend.append((j, uh))
                    if len(pend) > 1:
                        emit_1b(*pend.pop(0))
            while pend:
                emit_1b(*pend.pop(0))

        # ---------------- Phase 2: layer 2 (2A + 2B pipelined) ----------------
        with ExitStack() as ph:
            zps = ph.enter_context(tc.tile_pool(name="zps2", bufs=1, space="PSUM"))
            hps = ph.enter_context(tc.tile_pool(name="hps2", bufs=1, space="PSUM"))
            z2_pool = ph.enter_context(tc.tile_pool(name="z2p", bufs=2))

            z2t = {}
            pend = []

            def emit_2a(j, uh):
                zp = {(hk, q): zps.tile([128, 512], f32, name="zp2", tag=f"zp2{hk}{q}")
                      for hk in range(2) for q in range(2)}
                for ub in range(TK):
                    for hk in range(2):
                        lhs = h1[j][:, ub * H + hk * 128: ub * H + (hk + 1) * 128]
                        for q in range(2):
                            nc.tensor.matmul(
                                zp[(hk, q)][:], lhs,
                                adjt[ub][:, uh * 1024 + q * 512: uh * 1024 + (q + 1) * 512],
                                start=(ub == 0), stop=(ub == TK - 1))
                for hk in range(2):
                    zt = z2_pool.tile([128, 1024], bf16, name="z2t", tag=f"z2t{hk}")
                    z2t[(j, uh, hk)] = zt
                    nc.vector.tensor_copy(zt[:, 0:512], zp[(hk, 0)][:])
                    nc.vector.tensor_copy(zt[:, 512:1024], zp[(hk, 1)][:])

            def emit_2b(j, uh):
                for ho in range(2):
                    for q in range(2):
                        hp = hps.tile([128, 512], f32, name="hp2", tag=f"hp2{ho}{q}")
                        for hk in range(2):
                            nc.tensor.matmul(
                                hp[:], wg1_sb[:, hk * H + ho * 128: hk * H + (ho + 1) * 128],
                                z2t[(j, uh, hk)][:, q * 512:(q + 1) * 512],
                                start=(hk == 0), stop=False)
                        us = uh * 1024 + q * 512
                        nc.tensor.matmul(hp[:], bg1_sb[0:1, ho * 128:(ho + 1) * 128],
                                         rs_sb[0:1, us:us + 512], start=False, stop=True)
                        nc.vector.tensor_relu(h2t[j][:, ho * T + us: ho * T + us + 512],
                                              hp[:])

            for j in range(G):
                for uh in range(2):
                    emit_2a(j, uh)
                    pend.append((j, uh))
                    if len(pend) > 1:
                        emit_2b(*pend.pop(0))
            while pend:
                emit_2b(*pend.pop(0))

    # ---------------- Phases 3-4: the four LSTM passes ----------------
    with ExitStack() as ph:
        gps = ph.enter_context(tc.tile_pool(name="gps", bufs=1, space="PSUM"))
        gsb = ph.enter_context(tc.tile_pool(name="gsb", bufs=2))
        st_pool = ph.enter_context(tc.tile_pool(name="st", bufs=1))
        h_pool = ph.enter_context(tc.tile_pool(name="hs", bufs=2))
        gx_pool = ph.enter_context(tc.tile_pool(name="gx", bufs=2))

        c_st = [st_pool.tile([128, 1024], bf16, name=f"c_{j}", tag=f"c{j}")
                for j in range(G)]

        for p in range(1, MAX_SKIP + 1):
            nw = T // p
            wih = lw_pool.tile([128, 2 * 4 * H], bf16, name=f"wih{grp}{p}", tag="wih")
            for hk in range(2):
                nc.sync.dma_start(out=wih[:, hk * 4 * H:(hk + 1) * 4 * H],
                                  in_=io["wihT"][p - 1, hk * 128:(hk + 1) * 128, :])
            whh = None
            if p > 1:
                whh = lw_pool.tile([128, 2 * 4 * H], bf16, name=f"whh{grp}{p}", tag="whh")
                for hk in range(2):
                    nc.sync.dma_start(out=whh[:, hk * 4 * H:(hk + 1) * 4 * H],
                                      in_=io["whhT"][p - 1, hk * 128:(hk + 1) * 128, :])

            views = [[h2t[j][:, hk * T: hk * T + nw * p].rearrange(
                "a (w q) -> a w q", q=p) for hk in range(2)] for j in range(G)]

            for ws in range(0, nw, 512):
                ncw = min(512, nw - ws)
                spans = ([slice(0, 1024)] if ncw == 512
                         else [slice(0, ncw), slice(512, 512 + ncw)])
                h_t = [None] * G
                for t in range(p):
                    for j in range(G):
                        view = views[j]
                        if p > 1:
                            xc = gx_pool.tile([128, 1024], bf16, name="xc", tag="xc")
                            nc.gpsimd.tensor_copy(xc[:, 0:ncw],
                                                  view[0][:, ws:ws + ncw, t:t + 1])
                            nc.vector.tensor_copy(xc[:, 512:512 + ncw],
                                                  view[1][:, ws:ws + ncw, t:t + 1])
                        gates = "igo" if t == 0 else "ifgo"
                        gp = {}
                        # input-weight matmuls first (no state dependency)
                        for gn in gates:
                            psum = gps.tile([128, 1024], f32, name=f"ps_{gn}", tag=f"ps_{gn}")
                            gp[gn] = psum
                            for half in range(2):
                                jb = JB0[gn] + half
                                o = psum[:, half * 512: half * 512 + ncw]
                                for hk in range(2):
                                    rhs = (view[hk][:, ws:ws + ncw, 0:1] if p == 1
                                           else xc[:, hk * 512: hk * 512 + ncw])
                                    nc.tensor.matmul(
                                        o,
                                        wih[:, hk * 4 * H + jb * 128: hk * 4 * H + (jb + 1) * 128],
                                        rhs,
                                        start=(hk == 0),
                                        stop=(t == 0 and hk == 1))
                        if t > 0:
                            for gn in gates:
                                for half in range(2):
                                    jb = JB0[gn] + half
                                    o = gp[gn][:, half * 512: half * 512 + ncw]
                                    for hk in range(2):
                                        nc.tensor.matmul(
                                            o,
                                            whh[:, hk * 4 * H + jb * 128: hk * 4 * H + (jb + 1) * 128],
                                            h_t[j][:, hk * 512: hk * 512 + ncw],
                                            start=False, stop=(hk == 1))
                        act = {}
                        for gn in gates:
                            fn = AF.Tanh if gn == "g" else AF.Sigmoid
                            a = gsb.tile([128, 1024], bf16, name=f"a_{gn}", tag=f"a_{gn}")
                            act[gn] = a
                            for half in range(2):
                                col = (p - 1) * 8 + JB0[gn] + half
                                nc.scalar.activation(
                                    a[:, half * 512: half * 512 + ncw],
                                    gp[gn][:, half * 512: half * 512 + ncw],
                                    fn, bias=biasT_sb[:, col:col + 1])
                        cn = c_st[j]
                        if t == 0:
                            for s in spans:
                                nc.vector.tensor_mul(cn[:, s], act["i"][:, s], act["g"][:, s])
                        else:
                            for s in spans:
                                nc.vector.tensor_mul(act["g"][:, s], act["i"][:, s], act["g"][:, s])
                            for s in spans:
                                nc.gpsimd.tensor_mul(cn[:, s], act["f"][:, s], cn[:, s])
                            for s in spans:
                                nc.vector.tensor_add(cn[:, s], cn[:, s], act["g"][:, s])
                        # tanh(c) overwrites the i tile (free after c update)
                        tc_t = act["i"]
                        for s in spans:
                            nc.scalar.activation(tc_t[:, s], cn[:, s], AF.Tanh)
                        if t == p - 1:
                            for hk in range(2):
                                nc.vector.tensor_mul(
                                    view[hk][:, ws:ws + ncw, p - 1:p],
                                    act["o"][:, hk * 512: hk * 512 + ncw],
                                    tc_t[:, hk * 512: hk * 512 + ncw])
                        else:
                            hn = h_pool.tile([128, 1024], bf16, name="hn", tag=f"h{j}")
                            for s in spans:
                                nc.vector.tensor_mul(hn[:, s], act["o"][:, s], tc_t[:, s])
                            h_t[j] = hn

        # ------------- Phase 5: transpose h2T -> out (reuses gate PSUM) -------------
        osb = ph.enter_context(tc.tile_pool(name="osb", bufs=2))
        tptags = ["ps_i", "ps_f", "ps_g", "ps_o"]
        for j in range(G):
            b = bs + j
            for tg in range(4):
                tp = gps.tile([128, 1024], f32, name="tp", tag=tptags[tg])
                for q in range(4):
                    tk = tg * 4 + q
                    for hk in range(2):
                        nc.tensor.matmul(
                            tp[:, q * D + hk * 128: q * D + (hk + 1) * 128],
                            h2t[j][:, hk * T + tk * 128: hk * T + (tk + 1) * 128],
                            ident[:], start=True, stop=True)
                ot = osb.tile([128, 1024], f32, name="ot", tag="ot")
                if tg % 2 == 0:
                    nc.scalar.activation(ot[:], tp[:], AF.Copy)
                else:
                    nc.vector.tensor_copy(ot[:], tp[:])
                nc.sync.dma_start(
                    out=io["out"][b, tg * 512:(tg + 1) * 512, :].rearrange(
                        "(q p) d -> p q d", p=128),
                    in_=ot[:].rearrange("p (q d) -> p q d", d=D))


def _prep_host(inputs):
    import ml_dtypes
    bf16 = ml_dtypes.bfloat16

    x = np.asarray(inputs["x"], dtype=np.float32)
    adj = np.asarray(inputs["adj"], dtype=np.float32)
    adjT = np.ascontiguousarray(adj.T).astype(bf16)
    rs = np.ascontiguousarray(
        adj.sum(axis=1, dtype=np.float32).reshape(1, T)).astype(bf16)
    wg0 = np.ascontiguousarray(inputs["Wg0"], dtype=np.float32).astype(bf16)
    wg1 = np.ascontiguousarray(inputs["Wg1"], dtype=np.float32).astype(bf16)
    bg0 = np.ascontiguousarray(inputs["bg0"], dtype=np.float32).reshape(1, H).astype(bf16)
    bg1 = np.ascontiguousarray(inputs["bg1"], dtype=np.float32).reshape(1, H).astype(bf16)
    wihT = np.ascontiguousarray(
        np.asarray(inputs["Wih"], dtype=np.float32).transpose(0, 2, 1)).astype(bf16)
    whhT = np.ascontiguousarray(
        np.asarray(inputs["Whh"], dtype=np.float32).transpose(0, 2, 1)).astype(bf16)
    bias = np.asarray(inputs["bih"], dtype=np.float32) + np.asarray(inputs["bhh"], dtype=np.float32)
    biasT = np.ascontiguousarray(
        bias.reshape(MAX_SKIP, 8, 128).transpose(2, 0, 1).reshape(128, MAX_SKIP * 8))
    shared = dict(adjT=adjT, rs=rs, wg0=wg0, wg1=wg1, bg0=bg0, bg1=bg1,
                  wihT=wihT, whhT=whhT, biasT=biasT)
    xb = x.astype(bf16)
    in_maps = []
    for c in range(NCORES):
        m = dict(shared)
        m["x"] = np.ascontiguousarray(xb[c * BL:(c + 1) * BL])
        in_maps.append(m)
    return in_maps


def get_compiled():
    global _COMPILED
    if _COMPILED is None:
        _COMPILED = _build_program()
    return _COMPILED


def kernel(**inputs) -> np.ndarray:
    from concourse.bass_utils import run_bass_kernel_spmd

    nc = get_compiled()
    in_maps = _prep_host(inputs)
    res = run_bass_kernel_spmd(nc, in_maps, list(range(NCORES)))
    out = np.concatenate([res.results[c]["out"] for c in range(NCORES)], axis=0)
    return out.astype(np.float32)


# revision 20
# speedup vs baseline: 1.7432x; 1.2349x over previous
"""Trainium2 Bass kernel for the DTGL GCN+windowed-LSTM module (bf16 rewrite).

Computation (see reference):
  h = relu(adj @ (x @ Wg0 + bg0));  h = relu(adj @ (h @ Wg1 + bg1))
  for p in 1..4: run LSTM_p over disjoint length-p windows of h (zero init
  state), writing the last hidden state back at each window end (in place).

Sharding: pure data-parallel over batch B=64 across 8 cores (8 batches per
core); adj and all weights replicated. No collectives.

Perf design vs the fp32r baseline:
  - All matmul operands bf16 (PSUM accumulation stays f32): halves the
    moving-stream bytes, halves LDWEIGHTS time, halves SBUF/DMA footprint,
    and lowers PE power so the HAM clock-gate stays at full rate.
  - adjT is SBUF-resident (loaded once, bf16, 64KB/partition).
  - Software-pipelined phases: 1B(j-1) matmuls fill the PE pipe while
    1A(j)'s PSUM->SBUF copies drain (same for 2A/2B), so the PE never
    waits on a copy.
  - LSTM runs batches round-robin per timestep so PE matmuls of batch j+1
    overlap ACT/DVE/Pool cell math of batch j. Gate PSUM uses all 8 banks.
  - Output transpose via identity matmul (stationary = h2T block, moving =
    identity) producing f32 PSUM directly.
"""

import numpy as np

B, T, D, H = 64, 2048, 256, 256
MAX_SKIP = 4
NCORES = 8
BL = B // NCORES          # batches per core
G = 4                     # batches per group
NGRP = BL // G
TK = T // 128             # 16 t-chunks
JB0 = {"i": 0, "f": 2, "g": 4, "o": 6}
XS = 16.0                 # fp8 quantization scales: x, adjT, h1
AS = 8192.0
HS = 64.0

_COMPILED = None


def _build_program():
    import concourse.mybir as mybir
    import concourse.tile as tile
    from concourse import bacc

    f32 = mybir.dt.float32
    bf16 = mybir.dt.bfloat16
    f8 = mybir.dt.float8e4

    nc = bacc.Bacc("TRN2", target_bir_lowering=False, debug=False)

    io = dict(
        x=nc.dram_tensor("x", [BL, T, D], f8, kind="ExternalInput").ap(),
        adjT=nc.dram_tensor("adjT", [T, T], f8, kind="ExternalInput").ap(),
        rs=nc.dram_tensor("rs", [1, T], bf16, kind="ExternalInput").ap(),
        wg0=nc.dram_tensor("wg0", [D, H], bf16, kind="ExternalInput").ap(),
        wg1=nc.dram_tensor("wg1", [D, H], bf16, kind="ExternalInput").ap(),
        bg0=nc.dram_tensor("bg0", [1, H], bf16, kind="ExternalInput").ap(),
        bg1=nc.dram_tensor("bg1", [1, H], bf16, kind="ExternalInput").ap(),
        wihT=nc.dram_tensor("wihT", [MAX_SKIP, H, 4 * H], bf16, kind="ExternalInput").ap(),
        whhT=nc.dram_tensor("whhT", [MAX_SKIP, H, 4 * H], bf16, kind="ExternalInput").ap(),
        biasT=nc.dram_tensor("biasT", [128, MAX_SKIP * 8], f32, kind="ExternalInput").ap(),
        out=nc.dram_tensor("out", [BL, T, D], f32, kind="ExternalOutput").ap(),
    )

    with tile.TileContext(nc) as tc:
        _emit(nc, tc, mybir, io)

    nc.compile()
    return nc


def _emit(nc, tc, mybir, io):
    from contextlib import ExitStack
    from concourse.masks import make_identity

    f32 = mybir.dt.float32
    bf16 = mybir.dt.bfloat16
    f8 = mybir.dt.float8e4
    AF = mybir.ActivationFunctionType

    with ExitStack() as root:
        cp = root.enter_context(tc.tile_pool(name="const", bufs=1))
        # adjT resident: 8 pair-tiles [128, 2*2048] fp8 (ksub-plane-major) for
        # DoubleRow matmuls: plane ks covers rows (2m+ks)*128..(2m+ks+1)*128.
        adjt = []
        for m in range(TK // 2):
            a = cp.tile([128, 2 * T], f8, name=f"adjt_{m}")
            for ks in range(2):
                nc.sync.dma_start(
                    out=a[:, ks * T:(ks + 1) * T],
                    in_=io["adjT"][(2 * m + ks) * 128:(2 * m + ks + 1) * 128, :])
            adjt.append(a)
        wg0_sb = cp.tile([128, 2 * H], bf16, name="wg0_sb")
        wg1_sb = cp.tile([128, 2 * H], bf16, name="wg1_sb")
        for dk in range(2):
            nc.sync.dma_start(out=wg0_sb[:, dk * H:(dk + 1) * H],
                              in_=io["wg0"][dk * 128:(dk + 1) * 128, :])
            nc.sync.dma_start(out=wg1_sb[:, dk * H:(dk + 1) * H],
                              in_=io["wg1"][dk * 128:(dk + 1) * 128, :])
        bg0_sb = cp.tile([1, H], bf16, name="bg0_sb")
        bg1_sb = cp.tile([1, H], bf16, name="bg1_sb")
        rs_sb = cp.tile([1, T], bf16, name="rs_sb")
        biasT_sb = cp.tile([128, MAX_SKIP * 8], f32, name="biasT_sb")
        nc.sync.dma_start(out=bg0_sb[:], in_=io["bg0"][:])
        nc.sync.dma_start(out=bg1_sb[:], in_=io["bg1"][:])
        nc.sync.dma_start(out=rs_sb[:], in_=io["rs"][:])
        nc.sync.dma_start(out=biasT_sb[:], in_=io["biasT"][:])
        id32 = cp.tile([128, 128], f32, name="id32")
        ident = cp.tile([128, 128], bf16, name="ident")
        make_identity(nc, id32[:])
        nc.vector.tensor_copy(ident[:], id32[:])

        h2t_pool = root.enter_context(tc.tile_pool(name="h2tp", bufs=1))
        lw_pool = root.enter_context(tc.tile_pool(name="lw", bufs=2))

        for grp in range(NGRP):
            _group(nc, tc, io, f32, bf16, AF, grp, adjt, wg0_sb, wg1_sb,
                   bg0_sb, bg1_sb, rs_sb, biasT_sb, ident, h2t_pool, lw_pool)


def _group(nc, tc, io, f32, bf16, AF, grp, adjt, wg0_sb, wg1_sb, bg0_sb,
           bg1_sb, rs_sb, biasT_sb, ident, h2t_pool, lw_pool):
    from contextlib import ExitStack
    import concourse.mybir as mybir

    f8 = mybir.dt.float8e4
    DR = mybir.MatmulPerfMode.DoubleRow
    bs = grp * G
    # h2T slabs: feature-major [h(part within hk), hk*T + t], bf16.
    h2t = [h2t_pool.tile([128, 2 * T], bf16, name=f"h2t_{j}", tag=f"h2t_{j}")
           for j in range(G)]

    with ExitStack() as gcn:
        h1_pool = gcn.enter_context(tc.tile_pool(name="h1p", bufs=1))
        # h1 slabs: row-major [u(part within ub), ub*H + h], fp8 scaled by HS.
        h1 = [h1_pool.tile([128, TK * H], f8, name=f"h1_{j}", tag=f"h1_{j}")
              for j in range(G)]
        x_pool = gcn.enter_context(tc.tile_pool(name="xp", bufs=2))
        z1_pool = gcn.enter_context(tc.tile_pool(name="z1p", bufs=2))

        # ---------------- Phase 1: layer 1 (1A + 1B pipelined) ----------------
        with ExitStack() as ph:
            zps = ph.enter_context(tc.tile_pool(name="zps", bufs=1, space="PSUM"))
            hps = ph.enter_context(tc.tile_pool(name="hps", bufs=4, space="PSUM"))

            xs = []
            for j in range(G):
                xt = x_pool.tile([128, TK * D], f8, name=f"x_{j}", tag="xs")
                nc.sync.dma_start(
                    out=xt[:].rearrange("p (k d) -> p k d", d=D),
                    in_=io["x"][bs + j].rearrange("(k p) d -> p k d", p=128))
                xs.append(xt)

            z1t = {}   # (j, uh, dk) -> sbuf tile [128, 1024] bf16
            pend = []  # queue of emitted-1A halves awaiting 1B: (j, uh)

            def emit_1a(j, uh):
                zp = {(dk, q): zps.tile([128, 512], f32, name="zp", tag=f"zp{dk}{q}")
                      for dk in range(2) for q in range(2)}
                xv = xs[j][:].rearrange("p (k d) -> p k d", d=D)
                for m in range(TK // 2):
                    av = adjt[m][:].rearrange("p (k u) -> p k u", k=2)
                    for dk in range(2):
                        lhs = xv[:, 2 * m:2 * m + 2, dk * 128:(dk + 1) * 128]
                        for q in range(2):
                            us = uh * 1024 + q * 512
                            nc.tensor.matmul(
                                zp[(dk, q)][:], lhs, av[:, :, us:us + 512],
                                start=(m == 0), stop=(m == TK // 2 - 1),
                                perf_mode=DR)
                for dk in range(2):
                    zt = z1_pool.tile([128, 1024], bf16, name="z1t", tag=f"z1t{dk}")
                    z1t[(j, uh, dk)] = zt
                    nc.vector.tensor_scalar_mul(zt[:, 0:512], zp[(dk, 0)][:], 1.0 / (XS * AS))
                    nc.vector.tensor_scalar_mul(zt[:, 512:1024], zp[(dk, 1)][:], 1.0 / (XS * AS))

            def emit_1b(j, uh):
                for ub_l in range(8):
                    ub = uh * 8 + ub_l
                    hp = hps.tile([128, H], f32, name="hp", tag="hp")
                    for dk in range(2):
                        nc.tensor.matmul(
                            hp[:], z1t[(j, uh, dk)][:, ub_l * 128:(ub_l + 1) * 128],
                            wg0_sb[:, dk * H:(dk + 1) * H],
                            start=(dk == 0), stop=False)
                    nc.tensor.matmul(hp[:], rs_sb[0:1, ub * 128:(ub + 1) * 128],
                                     bg0_sb[0:1, :], start=False, stop=True)
                    # h1 = relu(HS * z) in fp8 (scale folded through relu)
                    nc.scalar.activation(h1[j][:, ub * H:(ub + 1) * H], hp[:],
                                         AF.Relu, scale=HS)

            for j in range(G):
                for uh in range(2):
                    emit_1a(j, uh)
                    pend.append((j, uh))
                    if len(pend) > 1:
                        emit_1b(*pend.pop(0))
            while pend:
                emit_1b(*pend.pop(0))

        # ---------------- Phase 2: layer 2 (2A + 2B pipelined) ----------------
        with ExitStack() as ph:
            zps = ph.enter_context(tc.tile_pool(name="zps2", bufs=1, space="PSUM"))
            hps = ph.enter_context(tc.tile_pool(name="hps2", bufs=1, space="PSUM"))
            z2_pool = ph.enter_context(tc.tile_pool(name="z2p", bufs=2))

            z2t = {}
            pend = []

            def emit_2a(j, uh):
                zp = {(hk, q): zps.tile([128, 512], f32, name="zp2", tag=f"zp2{hk}{q}")
                      for hk in range(2) for q in range(2)}
                hv = h1[j][:].rearrange("p (u h) -> p u h", h=H)
                for m in range(TK // 2):
                    av = adjt[m][:].rearrange("p (k u) -> p k u", k=2)
                    for hk in range(2):
                        lhs = hv[:, 2 * m:2 * m + 2, hk * 128:(hk + 1) * 128]
                        for q in range(2):
                            us = uh * 1024 + q * 512
                            nc.tensor.matmul(
                                zp[(hk, q)][:], lhs, av[:, :, us:us + 512],
                                start=(m == 0), stop=(m == TK // 2 - 1),
                                perf_mode=DR)
                for hk in range(2):
                    zt = z2_pool.tile([128, 1024], bf16, name="z2t", tag=f"z2t{hk}")
                    z2t[(j, uh, hk)] = zt
                    nc.vector.tensor_scalar_mul(zt[:, 0:512], zp[(hk, 0)][:], 1.0 / (HS * AS))
                    nc.vector.tensor_scalar_mul(zt[:, 512:1024], zp[(hk, 1)][:], 1.0 / (HS * AS))

            def emit_2b(j, uh):
                for ho in range(2):
                    for q in range(2):
                        hp = hps.tile([128, 512], f32, name="hp2", tag=f"hp2{ho}{q}")
                        for hk in range(2):
                            nc.tensor.matmul(
                                hp[:], wg1_sb[:, hk * H + ho * 128: hk * H + (ho + 1) * 128],
                                z2t[(j, uh, hk)][:, q * 512:(q + 1) * 512],
                                start=(hk == 0), stop=False)
                        us = uh * 1024 + q * 512
                        nc.tensor.matmul(hp[:], bg1_sb[0:1, ho * 128:(ho + 1) * 128],
                                         rs_sb[0:1, us:us + 512], start=False, stop=True)
                        nc.vector.tensor_relu(h2t[j][:, ho * T + us: ho * T + us + 512],
                                              hp[:])

            for j in range(G):
                for uh in range(2):
                    emit_2a(j, uh)
                    pend.append((j, uh))
                    if len(pend) > 1:
                        emit_2b(*pend.pop(0))
            while pend:
                emit_2b(*pend.pop(0))

    # ---------------- Phases 3-4: the four LSTM passes ----------------
    with ExitStack() as ph:
        gps = ph.enter_context(tc.tile_pool(name="gps", bufs=1, space="PSUM"))
        gsb = ph.enter_context(tc.tile_pool(name="gsb", bufs=2))
        st_pool = ph.enter_context(tc.tile_pool(name="st", bufs=1))
        h_pool = ph.enter_context(tc.tile_pool(name="hs", bufs=2))
        gx_pool = ph.enter_context(tc.tile_pool(name="gx", bufs=2))

        c_st = [st_pool.tile([128, 1024], bf16, name=f"c_{j}", tag=f"c{j}")
                for j in range(G)]

        for p in range(1, MAX_SKIP + 1):
            nw = T // p
            wih = lw_pool.tile([128, 2 * 4 * H], bf16, name=f"wih{grp}{p}", tag="wih")
            for hk in range(2):
                nc.sync.dma_start(out=wih[:, hk * 4 * H:(hk + 1) * 4 * H],
                                  in_=io["wihT"][p - 1, hk * 128:(hk + 1) * 128, :])
            whh = None
            if p > 1:
                whh = lw_pool.tile([128, 2 * 4 * H], bf16, name=f"whh{grp}{p}", tag="whh")
                for hk in range(2):
                    nc.sync.dma_start(out=whh[:, hk * 4 * H:(hk + 1) * 4 * H],
                                      in_=io["whhT"][p - 1, hk * 128:(hk + 1) * 128, :])

            views = [[h2t[j][:, hk * T: hk * T + nw * p].rearrange(
                "a (w q) -> a w q", q=p) for hk in range(2)] for j in range(G)]

            for ws in range(0, nw, 512):
                ncw = min(512, nw - ws)
                spans = ([slice(0, 1024)] if ncw == 512
                         else [slice(0, ncw), slice(512, 512 + ncw)])
                h_t = [None] * G
                for t in range(p):
                    for j in range(G):
                        view = views[j]
                        if p > 1:
                            xc = gx_pool.tile([128, 1024], bf16, name="xc", tag="xc")
                            nc.gpsimd.tensor_copy(xc[:, 0:ncw],
                                                  view[0][:, ws:ws + ncw, t:t + 1])
                            nc.vector.tensor_copy(xc[:, 512:512 + ncw],
                                                  view[1][:, ws:ws + ncw, t:t + 1])
                        gates = "igo" if t == 0 else "ifgo"
                        gp = {}
                        # input-weight matmuls first (no state dependency)
                        for gn in gates:
                            psum = gps.tile([128, 1024], f32, name=f"ps_{gn}", tag=f"ps_{gn}")
                            gp[gn] = psum
                            for half in range(2):
                                jb = JB0[gn] + half
                                o = psum[:, half * 512: half * 512 + ncw]
                                for hk in range(2):
                                    rhs = (view[hk][:, ws:ws + ncw, 0:1] if p == 1
                                           else xc[:, hk * 512: hk * 512 + ncw])
                                    nc.tensor.matmul(
                                        o,
                                        wih[:, hk * 4 * H + jb * 128: hk * 4 * H + (jb + 1) * 128],
                                        rhs,
                                        start=(hk == 0),
                                        stop=(t == 0 and hk == 1))
                        if t > 0:
                            for gn in gates:
                                for half in range(2):
                                    jb = JB0[gn] + half
                                    o = gp[gn][:, half * 512: half * 512 + ncw]
                                    for hk in range(2):
                                        nc.tensor.matmul(
                                            o,
                                            whh[:, hk * 4 * H + jb * 128: hk * 4 * H + (jb + 1) * 128],
                                            h_t[j][:, hk * 512: hk * 512 + ncw],
                                            start=False, stop=(hk == 1))
                        act = {}
                        for gn in gates:
                            fn = AF.Tanh if gn == "g" else AF.Sigmoid
                            a = gsb.tile([128, 1024], bf16, name=f"a_{gn}", tag=f"a_{gn}")
                            act[gn] = a
                            for half in range(2):
                                col = (p - 1) * 8 + JB0[gn] + half
                                nc.scalar.activation(
                                    a[:, half * 512: half * 512 + ncw],
                                    gp[gn][:, half * 512: half * 512 + ncw],
                                    fn, bias=biasT_sb[:, col:col + 1])
                        cn = c_st[j]
                        if t == 0:
                            for s in spans:
                                nc.vector.tensor_mul(cn[:, s], act["i"][:, s], act["g"][:, s])
                        else:
                            for s in spans:
                                nc.vector.tensor_mul(act["g"][:, s], act["i"][:, s], act["g"][:, s])
                            for s in spans:
                                nc.gpsimd.tensor_mul(cn[:, s], act["f"][:, s], cn[:, s])
                            for s in spans:
                                nc.vector.tensor_add(cn[:, s], cn[:, s], act["g"][:, s])
                        # tanh(c) overwrites the i tile (free after c update)
                        tc_t = act["i"]
                        for s in spans:
                            nc.scalar.activation(tc_t[:, s], cn[:, s], AF.Tanh)
                        if t == p - 1:
                            for hk in range(2):
                                nc.vector.tensor_mul(
                                    view[hk][:, ws:ws + ncw, p - 1:p],
                                    act["o"][:, hk * 512: hk * 512 + ncw],
                                    tc_t[:, hk * 512: hk * 512 + ncw])
                        else:
                            hn = h_pool.tile([128, 1024], bf16, name="hn", tag=f"h{j}")
                            for s in spans:
                                nc.vector.tensor_mul(hn[:, s], act["o"][:, s], tc_t[:, s])
                            h_t[j] = hn

        # ------------- Phase 5: transpose h2T -> out (reuses gate PSUM) -------------
        osb = ph.enter_context(tc.tile_pool(name="osb", bufs=2))
        tptags = ["ps_i", "ps_f", "ps_g", "ps_o"]
        for j in range(G):
            b = bs + j
            for tg in range(4):
                tp = gps.tile([128, 1024], f32, name="tp", tag=tptags[tg])
                for q in range(4):
                    tk = tg * 4 + q
                    for hk in range(2):
                        nc.tensor.matmul(
                            tp[:, q * D + hk * 128: q * D + (hk + 1) * 128],
                            h2t[j][:, hk * T + tk * 128: hk * T + (tk + 1) * 128],
                            ident[:], start=True, stop=True)
                ot = osb.tile([128, 1024], f32, name="ot", tag="ot")
                if tg % 2 == 0:
                    nc.scalar.activation(ot[:], tp[:], AF.Copy)
                else:
                    nc.vector.tensor_copy(ot[:], tp[:])
                nc.sync.dma_start(
                    out=io["out"][b, tg * 512:(tg + 1) * 512, :].rearrange(
                        "(q p) d -> p q d", p=128),
                    in_=ot[:].rearrange("p (q d) -> p q d", d=D))


def _prep_host(inputs):
    import ml_dtypes
    bf16 = ml_dtypes.bfloat16
    f8 = ml_dtypes.float8_e4m3fn

    def q8(a, scale):
        return np.ascontiguousarray(
            np.clip(np.asarray(a, dtype=np.float32) * scale, -240, 240)).astype(f8)

    x = np.asarray(inputs["x"], dtype=np.float32)
    adj = np.asarray(inputs["adj"], dtype=np.float32)
    adjT = q8(adj.T, AS)
    rs = np.ascontiguousarray(
        adj.sum(axis=1, dtype=np.float32).reshape(1, T)).astype(bf16)
    wg0 = np.ascontiguousarray(inputs["Wg0"], dtype=np.float32).astype(bf16)
    wg1 = np.ascontiguousarray(inputs["Wg1"], dtype=np.float32).astype(bf16)
    bg0 = np.ascontiguousarray(inputs["bg0"], dtype=np.float32).reshape(1, H).astype(bf16)
    bg1 = np.ascontiguousarray(inputs["bg1"], dtype=np.float32).reshape(1, H).astype(bf16)
    wihT = np.ascontiguousarray(
        np.asarray(inputs["Wih"], dtype=np.float32).transpose(0, 2, 1)).astype(bf16)
    whhT = np.ascontiguousarray(
        np.asarray(inputs["Whh"], dtype=np.float32).transpose(0, 2, 1)).astype(bf16)
    bias = np.asarray(inputs["bih"], dtype=np.float32) + np.asarray(inputs["bhh"], dtype=np.float32)
    biasT = np.ascontiguousarray(
        bias.reshape(MAX_SKIP, 8, 128).transpose(2, 0, 1).reshape(128, MAX_SKIP * 8))
    shared = dict(adjT=adjT, rs=rs, wg0=wg0, wg1=wg1, bg0=bg0, bg1=bg1,
                  wihT=wihT, whhT=whhT, biasT=biasT)
    xb = q8(x, XS)
    in_maps = []
    for c in range(NCORES):
        m = dict(shared)
        m["x"] = np.ascontiguousarray(xb[c * BL:(c + 1) * BL])
        in_maps.append(m)
    return in_maps


def get_compiled():
    global _COMPILED
    if _COMPILED is None:
        _COMPILED = _build_program()
    return _COMPILED


def kernel(**inputs) -> np.ndarray:
    from concourse.bass_utils import run_bass_kernel_spmd

    nc = get_compiled()
    in_maps = _prep_host(inputs)
    res = run_bass_kernel_spmd(nc, in_maps, list(range(NCORES)))
    out = np.concatenate([res.results[c]["out"] for c in range(NCORES)], axis=0)
    return out.astype(np.float32)
